# revision 1
# baseline (speedup 1.0000x reference)
"""DeformConv3D on 8 TRN2 cores: H-sharded, dense 5-tap tent-weight gather.

Per core (h-band of 12 output rows + halos):
  P1: offset conv (27 taps, K=64 matmuls accumulated in PSUM) -> off scratch DRAM
  P2: trilinear gather as separable 5-tap tent-weighted sums on DVE
      (one (b,c) plane per partition; all shifts are AP offsets into a
       padded per-plane window; tent weights vanish outside the clamp range
       so padded reads are weight-zero)
  P3: main conv + bias -> output h-band
"""
import sys, os
import numpy as np
from contextlib import ExitStack

sys.path.insert(0, "/opt/trn_rl_repo")
from concourse import bass, bacc, tile, mybir
from concourse.bass_utils import run_bass_kernel_spmd

F32 = mybir.dt.float32
BF16 = mybir.dt.bfloat16
ALU = mybir.AluOpType
AF = mybir.ActivationFunctionType

B, C, L, H, W = 2, 64, 16, 96, 96
CO1, CO2 = 192, 64
NCORES = 8
HB = H // NCORES       # 12 output rows per core
HW_ROWS = 20           # x window rows per core: [12k-4, 12k+16)
HG = 14                # gather rows per core: [12k-1, 12k+13)
NPP = HG * W           # 1344 gather outputs per (plane, l)
ZPAD, XPAD = 20, 100   # gather window padded dims (taps +-2)
WIN = HW_ROWS * ZPAD * XPAD
CZP, CXP = 18, 98      # conv window padded dims (taps +-1)
TAPS = (-2, -1, 0, 1, 2)

_nc1_cache = None
_nc2_cache = None


def build_program1():
    nc = bacc.Bacc("TRN2", target_bir_lowering=False, debug=False, num_devices=NCORES)
    xwin = nc.dram_tensor("xwin", [B, C, L, HW_ROWS, W], F32, kind="ExternalInput").ap()
    w_off = nc.dram_tensor("w_off", [27, C, CO1], F32, kind="ExternalInput").ap()
    off_scr = nc.dram_tensor("off_band", [B, CO1, L, HG, W], F32, kind="ExternalOutput").ap()
    ctx = ExitStack()
    with tile.TileContext(nc) as tc:
        # ---------------- Phase 1: offset conv ----------------
        with tc.tile_pool(name="p1", bufs=1) as p1, \
             tc.tile_pool(name="p1ps", bufs=2, space="PSUM") as p1ps, \
             tc.tile_pool(name="p1o", bufs=3) as p1o:
            wofft = p1.tile([C, 27, CO1], F32)
            nc.sync.dma_start(wofft[:], w_off.rearrange("t c m -> c t m"))
            for b in range(B):
                xc = p1.tile([C, CZP, HW_ROWS, CXP], F32, tag="xc")
                nc.vector.memset(xc[:].rearrange("c z y x -> c (z y x)"), 0.0)
                for z in range(L):
                    nc.sync.dma_start(xc[:, 1 + z, :, 1:W + 1], xwin[b, :, z])
                for l in range(L):
                    for hc0, hcn in ((0, 5), (5, 5), (10, 4)):
                        nmm = hcn * W
                        for m0, mw in ((0, 128), (128, 64)):
                            ps = p1ps.tile([128, 480], F32, tag="ps1")
                            for t in range(27):
                                dz, rem = divmod(t, 9)
                                dy, dx = divmod(rem, 3)
                                rhs = xc[:, l + dz,
                                         2 + hc0 + dy:2 + hc0 + dy + hcn,
                                         dx:dx + W]
                                nc.tensor.matmul(
                                    ps[:mw, :nmm], wofft[:, t, m0:m0 + mw],
                                    rhs, start=(t == 0), stop=(t == 26))
                            ob = p1o.tile([128, 480], F32, tag="ob1")
                            nc.vector.tensor_copy(ob[:mw, :nmm], ps[:mw, :nmm])
                            nc.sync.dma_start(
                                off_scr[b, m0:m0 + mw, l, hc0:hc0 + hcn, :]
                                .rearrange("m h x -> m (h x)"),
                                ob[:mw, :nmm])

    nc.finalize()
    return nc


def build_program2():
    nc = bacc.Bacc("TRN2", target_bir_lowering=False, debug=False, num_devices=NCORES)
    xwin = nc.dram_tensor("xwin", [B, C, L, HW_ROWS, W], BF16, kind="ExternalInput").ap()
    w_conv = nc.dram_tensor("w_conv", [27, C, CO2], F32, kind="ExternalInput").ap()
    b_conv = nc.dram_tensor("b_conv", [CO2, 1], F32, kind="ExternalInput").ap()
    offs = nc.dram_tensor("offs", [128, 3, L, NPP], F32, kind="ExternalInput").ap()
    grids = nc.dram_tensor("grids", [128, 1, NPP], F32, kind="ExternalInput").ap()
    out_ext = nc.dram_tensor("out", [B, CO2, L, HB, W], F32, kind="ExternalOutput").ap()
    def_scr = nc.dram_tensor("def_scr", [B, C, L, HG, W], F32).ap()
    ctx = ExitStack()
    with tile.TileContext(nc) as tc:
        # ---------------- Phase 2: tent gather ----------------
        with tc.tile_pool(name="p2w", bufs=1) as p2w, \
             tc.tile_pool(name="p2", bufs=1) as p2:
            win = p2w.tile([128, HW_ROWS, ZPAD, XPAD], BF16)
            nc.vector.memset(win[:].rearrange("p y z x -> p (y z x)"), 0.0)
            for b in range(B):
                for z in range(L):
                    nc.sync.dma_start(
                        win[64 * b:64 * b + 64, :, 2 + z, 2:W + 2],
                        xwin[b, :, z])
            gr = p2w.tile([128, 1, NPP], F32)
            nc.sync.dma_start(gr[:], grids)
            zbias = p2w.tile([128, 1], F32)
            nc.vector.memset(zbias[:], 0.0)

            for l in range(L):
                offc = p2.tile([128, 3, NPP], F32, tag="off")
                nc.sync.dma_start(offc[:], offs[:, :, l, :])
                az = offc[:, 0]
                ay = offc[:, 1]
                ax = offc[:, 2]

                # tent weights lam[dim][tap] = relu(1 - |a - t|)  (bf16)
                tneg = p2.tile([128, NPP], F32, tag="tneg")
                tpos = p2.tile([128, NPP], F32, tag="tpos")

                def tents(a, dst_tag, taps):
                    row = []
                    for t in taps:
                        nc.vector.tensor_scalar(tpos[:], a, 1.0 - float(t), None, ALU.add)
                        nc.vector.tensor_scalar(tneg[:], a, -1.0, 1.0 + float(t), ALU.mult, ALU.add)
                        nc.vector.tensor_tensor(tpos[:], tpos[:], tneg[:], ALU.min)
                        lt = p2.tile([128, NPP], BF16, tag=f"{dst_tag}_{t}")
                        nc.scalar.activation(lt[:], tpos[:], AF.Relu, bias=zbias[:])
                        row.append(lt)
                    return row

                lamx = tents(ax, "lamx", TAPS)
                lamy = tents(ay, "lamy", TAPS)

                acc = p2.tile([128, NPP], F32, tag="acc")
                tmpi = p2.tile([128, NPP], F32, tag="tmpi")
                tmpb = p2.tile([128, NPP], F32, tag="tmpb")
                prod = p2.tile([128, NPP], BF16, tag="prod")
                lam = [None, lamy, lamx]
                for iz, sz in enumerate(TAPS):
                    lamz = tents(az, "lamz", (sz,))[0]
                    for iy, sy in enumerate(TAPS):
                        for ix, sx in enumerate(TAPS):
                            v = win[:, 3 + sy:3 + sy + HG,
                                    l + 2 + sz,
                                    2 + sx:2 + sx + W]
                            if ix == 0:
                                nc.vector.tensor_tensor(tmpi[:], lam[2][0][:], v, ALU.mult)
                            else:
                                nc.vector.tensor_tensor(prod[:], lam[2][ix][:], v, ALU.mult)
                                nc.vector.tensor_tensor(tmpi[:], tmpi[:], prod[:], ALU.add)
                        if iy == 0:
                            nc.vector.tensor_tensor(tmpb[:], lam[1][0][:], tmpi[:], ALU.mult)
                        else:
                            nc.vector.tensor_tensor(tmpi[:], lam[1][iy][:], tmpi[:], ALU.mult)
                            nc.vector.tensor_tensor(tmpb[:], tmpb[:], tmpi[:], ALU.add)
                    if iz == 0:
                        nc.vector.tensor_tensor(acc[:], lamz[:], tmpb[:], ALU.mult)
                    else:
                        nc.vector.tensor_tensor(tmpb[:], lamz[:], tmpb[:], ALU.mult)
                        nc.vector.tensor_tensor(acc[:], acc[:], tmpb[:], ALU.add)
                # zero rows whose global h is outside [0, 96)
                nc.vector.tensor_tensor(acc[:], acc[:], gr[:, 0], ALU.mult)
                for b in range(B):
                    nc.sync.dma_start(
                        def_scr[b, :, l].rearrange("c h x -> c (h x)"),
                        acc[64 * b:64 * b + 64, :])

        # ---------------- Phase 3: main conv ----------------
        with tc.tile_pool(name="p3", bufs=1) as p3, \
             tc.tile_pool(name="p3ps", bufs=2, space="PSUM") as p3ps, \
             tc.tile_pool(name="p3o", bufs=3) as p3o:
            wct = p3.tile([C, 27, CO2], F32)
            nc.sync.dma_start(wct[:], w_conv.rearrange("t c m -> c t m"))
            bct = p3.tile([CO2, 1], F32)
            nc.sync.dma_start(bct[:], b_conv)
            for b in range(B):
                dc = p3.tile([C, CZP, HG + 2, CXP], F32, tag="dc")
                nc.vector.memset(dc[:].rearrange("c z y x -> c (z y x)"), 0.0)
                for z in range(L):
                    nc.sync.dma_start(dc[:, 1 + z, 1:HG + 1, 1:W + 1], def_scr[b, :, z])
                for l in range(L):
                    for hc0, hcn in ((0, 5), (5, 5), (10, 2)):
                        nmm = hcn * W
                        ps = p3ps.tile([CO2, 480], F32, tag="ps3")
                        for t in range(27):
                            dz, rem = divmod(t, 9)
                            dy, dx = divmod(rem, 3)
                            # out row r=4+hc0+j -> dc y index r+dy-3
                            rhs = dc[:, l + dz,
                                     1 + hc0 + dy:1 + hc0 + dy + hcn,
                                     dx:dx + W]
                            nc.tensor.matmul(
                                ps[:, :nmm], wct[:, t, :],
                                rhs, start=(t == 0), stop=(t == 26))
                        ob = p3o.tile([CO2, 480], F32, tag="ob3")
                        nc.vector.tensor_scalar(ob[:, :nmm], ps[:, :nmm], bct[:], None, ALU.add)
                        nc.sync.dma_start(
                            out_ext[b, :, l, hc0:hc0 + hcn, :]
                            .rearrange("m h x -> m (h x)"),
                            ob[:, :nmm])
    nc.finalize()
    return nc


def kernel(x, w_off, w_conv, b_conv):
    global _nc1_cache, _nc2_cache
    x = np.asarray(x, dtype=np.float32)
    w_off = np.asarray(w_off, dtype=np.float32)
    w_conv = np.asarray(w_conv, dtype=np.float32)
    b_conv = np.asarray(b_conv, dtype=np.float32)

    if _nc1_cache is None:
        _nc1_cache = build_program1()
        _nc2_cache = build_program2()

    xp = np.zeros((B, C, L, H + 8, W), np.float32)
    xp[:, :, :, 4:4 + H, :] = x
    wofft = np.ascontiguousarray(
        w_off.reshape(CO1, C, 27).transpose(2, 1, 0))        # [27, C, CO1]
    wct = np.ascontiguousarray(
        w_conv.reshape(CO2, C, 27).transpose(2, 1, 0))       # [27, C, CO2]
    bc = np.ascontiguousarray(b_conv.reshape(CO2, 1))

    xwins = [np.ascontiguousarray(xp[:, :, :, 12 * k:12 * k + HW_ROWS, :])
             for k in range(NCORES)]
    import ml_dtypes
    xwins_bf = [w.astype(ml_dtypes.bfloat16) for w in xwins]
    in1 = [{"xwin": xwins[k], "w_off": wofft} for k in range(NCORES)]
    res1 = run_bass_kernel_spmd(_nc1_cache, in1, list(range(NCORES)))

    # reassemble full off field from per-core bands (band rows = 12k-1..12k+13)
    off_full = np.empty((B, CO1, L, H, W), np.float32)
    for k in range(NCORES):
        band = res1.results[k]["off_band"]
        off_full[:, :, :, 12 * k:12 * k + HB, :] = band[:, :, :, 1:1 + HB, :]
    # contiguous-view scramble: plane (b,c) offsets at spatial p, comp k =
    # flat element 3p+k of its 3-channel block. Per (l, h) row that is a
    # contiguous 288-float run, so a padded reshape + slice does it all.
    tri = off_full.reshape(B * C, L, 3 * H * W)
    trip = np.zeros((B * C, L, 3 * (H + 2) * W), np.float32)
    trip[:, :, 3 * W:3 * (H + 1) * W] = tri            # one pad row each side
    trip = trip.reshape(B * C, L, H + 2, W * 3)
    in2 = []
    gy = np.repeat(np.arange(HG, dtype=np.float32) + 3.0, W)
    gx = np.tile(np.arange(W, dtype=np.float32), HG)
    lgrid = np.arange(L, dtype=np.float32)[None, None, :, None]
    for k in range(NCORES):
        seg = trip[:, :, 12 * k:12 * k + HG, :]        # rows 12k-1..12k+13
        offs = np.ascontiguousarray(
            seg.reshape(128, L, HG * W, 3).transpose(0, 3, 1, 2))
        # displacements a = clamp(off + grid) - grid, computed on host
        hglobf = np.repeat(np.arange(HG, dtype=np.float32) + (12 * k - 1), W)
        offs[:, 0] = np.clip(offs[:, 0] + lgrid[0], 0.0, 15.0) - lgrid[0]
        offs[:, 1] = (np.clip(offs[:, 1] + hglobf[None, None, :], 0.0, 95.0)
                      - hglobf[None, None, :])
        offs[:, 2] = (np.clip(offs[:, 2] + gx[None, None, :], 0.0, 95.0)
                      - gx[None, None, :])
        hglob = np.repeat(np.arange(HG) + (12 * k - 1), W)
        ymask = ((hglob >= 0) & (hglob < H)).astype(np.float32)
        grids = np.broadcast_to(ymask[None, None], (128, 1, NPP)).copy()
        in2.append({
            "xwin": xwins_bf[k], "w_conv": wct, "b_conv": bc,
            "offs": offs,
            "grids": grids,
        })
    res2 = run_bass_kernel_spmd(_nc2_cache, in2, list(range(NCORES)))
    out = np.empty((B, CO2, L, H, W), np.float32)
    for k in range(NCORES):
        out[:, :, :, 12 * k:12 * k + HB, :] = res2.results[k]["out"]
    return out



# revision 2
# speedup vs baseline: 1.1947x; 1.1947x over previous
"""DeformConv3D on 8 TRN2 cores — SINGLE fused launch.

Per core (h-band of 12 output rows):
  P1': offset conv computed directly in the torch-contiguous-view
       scrambled arrangement this core's gather needs: for each target
       depth l, 3 fixed segments (15/12/15 source rows) select channel
       group j=(3l+s)//16 and source depth l'=(3l+s)%16 via
       host-permuted input slabs (xa) + host-sliced weights (wsched).
       Output = flat [42*96] per plane; component k of the gather
       coords is the stride-3 slice [3n+k].
  P2: clamp coords on-device, separable 5-tap tent-weight gather (DVE)
  P3: main 3x3x3 conv + bias on the 12-row band (bf16 matmuls)
No host round-trip, no collectives: one launch instead of two.
"""
import sys
import numpy as np

sys.path.insert(0, "/opt/trn_rl_repo")
from concourse import bass, bacc, tile, mybir
from concourse.bass_utils import run_bass_kernel_spmd

F32 = mybir.dt.float32
BF16 = mybir.dt.bfloat16
ALU = mybir.AluOpType
AF = mybir.ActivationFunctionType

B, C, L, H, W = 2, 64, 16, 96, 96
NCORES = 8
HB = 12                 # output rows per core
HG = 14                 # band rows (12k-1 .. 12k+12)
NPP = HG * W            # 1344
TAPS = (-2, -1, 0, 1, 2)
SEGS = [(0, 5), (5, 4), (9, 5)]          # (band-row start, n band rows)
SEG_RBASE = (0, 17, 31)                  # xa row offset per segment
SEG_COL0 = (0, 1440, 2592)               # src column offset per segment
# P1' psum chunks per segment: (chunk src-row start, n src rows)
SEG_CHUNKS = ([(0, 5), (5, 5), (10, 5)],
              [(0, 4), (4, 4), (8, 4)],
              [(0, 5), (5, 5), (10, 5)])
LCH = 2                 # target-l chunk size for the gather window

_nc_cache = None


def core_sched(k):
    rows = [12 * k - 1 + i for i in range(HG)]
    sm = []
    for h in rows:
        s = 0 if h < 0 else (2 if h > 95 else h // 32)
        sm.append((s, h - 32 * s))
    sched = []
    for (i0, nr) in SEGS:
        s, m32_0 = sm[i0]
        for t in range(nr):
            assert sm[i0 + t] == (s, m32_0 + t)
        sched.append((s, 3 * m32_0))
    return sched


def build_fused():
    nc = bacc.Bacc("TRN2", target_bir_lowering=False, debug=False,
                   num_devices=NCORES)
    xwin = nc.dram_tensor("xwin", [B, C, L, 20, W], BF16,
                          kind="ExternalInput").ap()
    xa = nc.dram_tensor("xa", [B, C, L, 3, 48, 98], BF16,
                        kind="ExternalInput").ap()
    wsched = nc.dram_tensor("wsched", [128, L, 3, 27, 64], BF16,
                            kind="ExternalInput").ap()
    wct_in = nc.dram_tensor("wct", [64, 27, 64], BF16,
                            kind="ExternalInput").ap()
    bct_in = nc.dram_tensor("bct", [64, 1], F32, kind="ExternalInput").ap()
    bnds_in = nc.dram_tensor("bnds", [128, 5, NPP], BF16,
                             kind="ExternalInput").ap()
    out_ext = nc.dram_tensor("out", [B, 64, L, HB, W], F32,
                             kind="ExternalOutput").ap()
    def_scr = nc.dram_tensor("def_scr", [B, C, L, HG, W], BF16).ap()

    # const APs for activation biases (-t for tent taps); 0.0/1.0 built in
    for v in (2.0, -1.0, -2.0):
        t_ = nc.alloc_sbuf_tensor(f"cstb{int(v*10)}", [128, 1], F32)
        nc.gpsimd.memset(t_.ap(), v)
        nc.const_aps.aps[(F32, v)] = t_.ap()
    nc.all_engine_barrier()

    with tile.TileContext(nc) as tc:
        with tc.tile_pool(name="pp", bufs=1) as pp, \
             tc.tile_pool(name="pa", bufs=2) as pa, \
             tc.tile_pool(name="pc", bufs=1) as pc, \
             tc.tile_pool(name="psp", bufs=4, space="PSUM") as psp:
            bndst = pp.tile([128, 5, NPP], BF16)
            nc.sync.dma_start(bndst[:], bnds_in)
            win = pp.tile([128, 20, LCH + 4, 100], BF16)

            for lc0 in range(0, L, LCH):
                # (re)load gather window depths lc0-2 .. lc0+LCH+1
                nc.vector.memset(
                    win[:].rearrange("p y z x -> p (y z x)"), 0.0)
                for b in range(B):
                    for z in range(lc0 - 2, lc0 + LCH + 2):
                        if 0 <= z < L:
                            nc.sync.dma_start(
                                win[64 * b:64 * b + 64, :, z - lc0 + 2, 2:98],
                                xwin[b, :, z])
                for l in range(lc0, lc0 + LCH):
                    # ---- P1': scrambled offset conv ----
                    xat = pa.tile([128, 3, 48, 98], BF16, tag="xa")
                    for b in range(B):
                        nc.sync.dma_start(xat[64 * b:64 * b + 64], xa[b, :, l])
                    wst = pa.tile([128, 3, 27, 64], BF16, tag="wst")
                    nc.sync.dma_start(wst[:], wsched[:, l])
                    src = pa.tile([128, 4032], BF16, tag="src")
                    for b in range(B):
                        for seg in range(3):
                            rb = SEG_RBASE[seg]
                            for (cr0, cn) in SEG_CHUNKS[seg]:
                                nmm = cn * 96
                                ps = psp.tile([64, 480], F32, tag="ps1")
                                for t in range(27):
                                    dz, rem = divmod(t, 9)
                                    dy, dx = divmod(rem, 3)
                                    rhs = xat[64 * b:64 * b + 64, dz,
                                              rb + cr0 + dy:rb + cr0 + dy + cn,
                                              dx:dx + 96]
                                    nc.tensor.matmul(
                                        ps[:, :nmm], wst[64 * b:64 * b + 64, seg, t, :], rhs,
                                        start=(t == 0), stop=(t == 26))
                                col0 = SEG_COL0[seg] + cr0 * 96
                                nc.scalar.activation(
                                    src[64 * b:64 * b + 64,
                                        col0:col0 + nmm],
                                    ps[:, :nmm], AF.Copy)
                    # ---- deinterleave + clamp ----
                    sr = src[:].rearrange("p (n t) -> p t n", t=3)
                    azc = pc.tile([128, NPP], F32, tag="azc")
                    ayc = pc.tile([128, NPP], F32, tag="ayc")
                    axc = pc.tile([128, NPP], F32, tag="axc")
                    nc.vector.tensor_scalar(azc[:], sr[:, 0], float(-l),
                                            None, ALU.max)
                    nc.vector.tensor_scalar(azc[:], azc[:], float(15 - l),
                                            None, ALU.min)
                    nc.vector.tensor_tensor(ayc[:], sr[:, 1], bndst[:, 1],
                                            ALU.min)
                    nc.vector.tensor_tensor(ayc[:], ayc[:], bndst[:, 0],
                                            ALU.max)
                    nc.vector.tensor_tensor(axc[:], sr[:, 2], bndst[:, 3],
                                            ALU.min)
                    nc.vector.tensor_tensor(axc[:], axc[:], bndst[:, 2],
                                            ALU.max)
                    # ---- tent weights for x, y ----
                    lamx, lamy = [], []
                    for nm, a, row in (("lx", axc, lamx), ("ly", ayc, lamy)):
                        for i, t in enumerate(TAPS):
                            u = pc.tile([128, NPP], F32,
                                        tag="tmpi" if i % 2 == 0 else "tmpb")
                            nc.scalar.activation(u[:], a[:], AF.Abs,
                                                 bias=float(-t), scale=1.0)
                            lt = pc.tile([128, NPP], BF16, tag=f"{nm}{i}")
                            nc.scalar.activation(lt[:], u[:], AF.Relu,
                                                 bias=1.0, scale=-1.0)
                            row.append(lt)
                    # ---- gather: separable tent sums ----
                    acc = pc.tile([128, NPP], F32, tag="acc")
                    tmpi = pc.tile([128, NPP], F32, tag="tmpi")
                    tmpb = pc.tile([128, NPP], F32, tag="tmpb")
                    prod = pc.tile([128, NPP], BF16, tag="prod")
                    uz = pc.tile([128, NPP], F32, tag="uz")
                    lamz = pc.tile([128, NPP], BF16, tag="lz")
                    for iz, sz in enumerate(TAPS):
                        nc.scalar.activation(uz[:], azc[:], AF.Abs,
                                             bias=float(-sz), scale=1.0)
                        nc.scalar.activation(lamz[:], uz[:], AF.Relu,
                                             bias=1.0, scale=-1.0)
                        zi = l - lc0 + 2 + sz
                        for iy, sy in enumerate(TAPS):
                            for ix, sx in enumerate(TAPS):
                                v = win[:, 3 + sy:3 + sy + HG, zi,
                                        2 + sx:2 + sx + W]
                                if ix == 0:
                                    nc.vector.tensor_tensor(
                                        tmpi[:], lamx[0][:], v, ALU.mult)
                                else:
                                    nc.vector.tensor_tensor(
                                        prod[:], lamx[ix][:], v, ALU.mult)
                                    nc.vector.tensor_tensor(
                                        tmpi[:], tmpi[:], prod[:], ALU.add)
                            if iy == 0:
                                nc.vector.tensor_tensor(
                                    tmpb[:], lamy[0][:], tmpi[:], ALU.mult)
                            else:
                                nc.vector.tensor_tensor(
                                    tmpi[:], lamy[iy][:], tmpi[:], ALU.mult)
                                nc.vector.tensor_tensor(
                                    tmpb[:], tmpb[:], tmpi[:], ALU.add)
                        if iz == 0:
                            nc.vector.tensor_tensor(
                                acc[:], lamz[:], tmpb[:], ALU.mult)
                        else:
                            nc.vector.tensor_tensor(
                                tmpb[:], lamz[:], tmpb[:], ALU.mult)
                            nc.vector.tensor_tensor(
                                acc[:], acc[:], tmpb[:], ALU.add)
                    accb = pc.tile([128, NPP], BF16, tag="accb")
                    nc.vector.tensor_tensor(accb[:], acc[:], bndst[:, 4],
                                            ALU.mult)
                    for b in range(B):
                        nc.sync.dma_start(
                            def_scr[b, :, l].rearrange("c h x -> c (h x)"),
                            accb[64 * b:64 * b + 64, :])

        # ---------------- P3: main conv + bias ----------------
        with tc.tile_pool(name="p3", bufs=1) as p3, \
             tc.tile_pool(name="p3ps", bufs=4, space="PSUM") as p3ps, \
             tc.tile_pool(name="p3o", bufs=4) as p3o:
            wctt = p3.tile([64, 27, 64], BF16)
            nc.sync.dma_start(wctt[:], wct_in)
            bctt = p3.tile([64, 1], F32)
            nc.sync.dma_start(bctt[:], bct_in)
            for b in range(B):
                dcb = p3.tile([64, 18, HG, 98], BF16, tag="dcb")
                nc.vector.memset(
                    dcb[:].rearrange("c z y x -> c (z y x)"), 0.0)
                for z in range(L):
                    nc.sync.dma_start(dcb[:, 1 + z, :, 1:97],
                                      def_scr[b, :, z])
                for l in range(L):
                    for hc0, hcn in ((0, 5), (5, 5), (10, 2)):
                        nmm = hcn * 96
                        ps = p3ps.tile([64, 480], F32, tag="ps3")
                        for t in range(27):
                            dz, rem = divmod(t, 9)
                            dy, dx = divmod(rem, 3)
                            rhs = dcb[:, l + dz, hc0 + dy:hc0 + dy + hcn,
                                      dx:dx + 96]
                            nc.tensor.matmul(ps[:, :nmm], wctt[:, t, :], rhs,
                                             start=(t == 0), stop=(t == 26))
                        ob = p3o.tile([64, 480], F32, tag="ob")
                        nc.vector.tensor_scalar(ob[:, :nmm], ps[:, :nmm],
                                                bctt[:], None, ALU.add)
                        nc.sync.dma_start(
                            out_ext[b, :, l, hc0:hc0 + hcn, :]
                            .rearrange("m h x -> m (h x)"),
                            ob[:, :nmm])
    nc.finalize()
    return nc


def kernel(x, w_off, w_conv, b_conv):
    global _nc_cache
    import ml_dtypes
    bf16 = ml_dtypes.bfloat16
    x = np.asarray(x, dtype=np.float32)
    w_off = np.asarray(w_off, dtype=np.float32)
    w_conv = np.asarray(w_conv, dtype=np.float32)
    b_conv = np.asarray(b_conv, dtype=np.float32)

    if _nc_cache is None:
        _nc_cache = build_fused()

    # padded x: depths +-1 (idx=1+l), rows +-4 (idx=4+h), w +-1 (idx=1+w)
    xp = np.zeros((B, C, 18, 104, 98), bf16)
    xp[:, :, 1:17, 4:100, 1:97] = x

    w3 = w_off.reshape(64, 3, 64, 27)        # [cp, j, ci, tap]
    wct = np.ascontiguousarray(
        w_conv.reshape(64, 64, 27).transpose(1, 2, 0)).astype(bf16)
    bct = np.ascontiguousarray(b_conv.reshape(64, 1))

    gxr = np.tile(np.arange(W, dtype=np.float32), HG)
    in_maps = []
    for k in range(NCORES):
        sched = core_sched(k)
        xwin_k = np.ascontiguousarray(
            xp[:, :, 1:17, 12 * k:12 * k + 20, 1:97])
        xa_k = np.empty((B, C, L, 3, 48, 98), bf16)
        ws_k = np.empty((128, L, 3, 27, 64), bf16)
        for l in range(L):
            for seg, ((i0, nr), (s, r0)) in enumerate(zip(SEGS, sched)):
                j, lp = divmod(3 * l + s, 16)
                rb = SEG_RBASE[seg]
                xa_k[:, :, l, :, rb:rb + 3 * nr + 2, :] = \
                    xp[:, :, lp:lp + 3, 4 + r0 - 1:4 + r0 + 3 * nr + 1, :]
                ws_k[:64, l, seg] = w3[:, j].transpose(1, 2, 0).astype(bf16)
                ws_k[64:, l, seg] = ws_k[:64, l, seg]
        ghr = np.repeat(np.arange(HG, dtype=np.float32) + (12 * k - 1), W)
        ymask = ((ghr >= 0) & (ghr <= 95)).astype(np.float32)
        bnds = np.broadcast_to(
            np.stack([-ghr, 95.0 - ghr, -gxr, 95.0 - gxr, ymask])[None],
            (128, 5, NPP)).astype(bf16)
        in_maps.append({
            "xwin": xwin_k, "xa": np.ascontiguousarray(xa_k),
            "wsched": np.ascontiguousarray(ws_k),
            "wct": wct, "bct": bct,
            "bnds": np.ascontiguousarray(bnds),
        })
    res = run_bass_kernel_spmd(_nc_cache, in_maps, list(range(NCORES)))
    out = np.empty((B, 64, L, H, W), np.float32)
    for k in range(NCORES):
        out[:, :, :, 12 * k:12 * k + HB, :] = res.results[k]["out"]
    return out
